# revision 28
# baseline (speedup 1.0000x reference)
# Trainium2 Bass kernel for nn_Decoder_14568529068506 (gnn_message_passing).
#
# Reference computation (per scene s of 32, P=48 peds):
#   rel[i,j]  = obs[j] - obs[i]                  (P,P,2T)   2T=16
#   emb       = rel @ W_se.T                     (P,P,512)
#   emb      *= tile(traj_weight[s])             (P,P,512)
#   x         = concat([emb, h[j]], -1)          (P,P,576)
#   x1        = relu(x @ W1.T + b1)              (P,P,512)
#   x2        = relu(x1 @ W2.T + b2)             (P,P,1024)
#   out[s,i]  = max_j x2[i,j]                    (P,1024)
#
# Kernel restructuring (validated exactly in fp32 numpy):
#  * The tiled traj_weight multiply + spatial embedding + W1 are fused:
#      out1[d,row] = sum_{(ct,g)} Wf[d,(ct,g)] * tw[row,ct] * rel[row,g]
#    with Wf[d, ct*16+g] = sum_{k%2==c} W1[d, t*64+k] * W_se[t*64+k, g].
#    So MLP1 contracts over 256 "rel2" features (+64 h features) instead
#    of 576, and the (P,P,512) embedding is never materialized.
#  * rel2 = tw_rep * rel_rep is built feature-major on 128 partitions:
#      rel_rep = obs_rep.T @ D   (D = +-1 pairwise difference matrix)
#    and tw_rep comes pre-replicated from the host via DMA (bandwidth is
#    cheap here; the replication matmuls caused PE pipeline stalls).
#  * ALL matmuls are full K=128 (zero-padded where needed): quadrant
#    (row_grp/tile_position) matmuls serialize LDWEIGHTS behind the
#    draining matmul (~+100ns each); full-tile matmuls chain at
#    163ns/384cols = 2.4GHz.
#  * relu/bias commute with max-pool, so MLP2 outputs are max-pooled
#    straight out of PSUM (reduces split across DVE and Pool engines);
#    bias+relu are applied post-pool on the Scalar engine.
#  * Matmuls run in bf16. PSUM accumulation stays fp32.
#
# Sharding: scenes are data-parallel across the 8 cores (4 scenes each);
# weights replicated; the (192,1024) per-core outputs are concatenated on
# the host (no collectives needed).

import numpy as np

S, P, T, E, H = 32, 48, 8, 64, 64
D1, D2 = 512, 1024
B = S * P
NCORES = 8
SC = S // NCORES          # scenes per core
NB = 6                    # row blocks per scene
NBLK = P * P // NB        # 384 columns (pairs) per block = 8 i-groups x 48 j
IB = NBLK // P            # i-groups per block (8)
S_W = 16.0                # fp8 scale on W2 (unapplied post-pool)


def _host_constants(W_se, W1, W2, b1, b2):
    """Precompute fused weights + structural constant matrices (fp32)."""
    W_se = np.asarray(W_se, np.float32)
    W1 = np.asarray(W1, np.float32)
    W2 = np.asarray(W2, np.float32)
    b1 = np.asarray(b1, np.float32)
    b2 = np.asarray(b2, np.float32)

    W1e, W1h = W1[:, :512], W1[:, 512:]
    Wf = np.zeros((D1, 256), np.float32)
    for c in range(2):
        for t in range(T):
            ct = c * 8 + t
            f = t * 64 + np.arange(c, 64, 2)
            Wf[:, ct * 16:(ct + 1) * 16] = W1e[:, f] @ W_se[f, :]

    # pairwise difference matrix, K zero-padded to 128
    Dm = np.zeros((128, P * P), np.float32)
    ii, jj = np.meshgrid(np.arange(P), np.arange(P), indexing="ij")
    rows = (ii * P + jj).ravel()
    np.add.at(Dm, (jj.ravel(), rows), 1.0)
    np.add.at(Dm, (ii.ravel(), rows), -1.0)

    # lhsT tile layouts: [128, kTiles, M] so DMAs are contiguous
    Wf_sb = np.ascontiguousarray(Wf.T.reshape(2, 128, D1).transpose(1, 0, 2))
    W1h_sb = np.zeros((128, D1), np.float32)
    W1h_sb[:H, :] = W1h.T                                    # K-padded
    # W2 as scaled fp8 hi+lo (e4m3 bit patterns match TRN FP8_EXP4 up to
    # +-240; the lo term makes the weight-side quantization ~bf16-exact)
    import ml_dtypes
    f8 = ml_dtypes.float8_e4m3fn
    W2s = np.ascontiguousarray(W2.T.reshape(4, 128, D2).transpose(1, 0, 2)) * S_W
    W2h = np.clip(W2s, -240, 240).astype(f8)
    W2l = np.clip(W2s - W2h.astype(np.float32), -240, 240).astype(f8)
    b1_sb = np.ascontiguousarray(b1.reshape(4, 128).T)       # (128, 4)
    b2_sb = np.ascontiguousarray(b2.reshape(8, 128).T)       # (128, 8)
    ident = np.eye(128, dtype=np.float32)
    return dict(Wf_sb=Wf_sb, W1h_sb=W1h_sb, W2h_sb=W2h, W2l_sb=W2l,
                b1_sb=b1_sb, b2_sb=b2_sb, Dm=Dm, ident=ident)


def build_program(n_scenes=SC):
    """Emit the per-core Bass/Tile program. Returns the compiled Bacc."""
    from contextlib import ExitStack
    import concourse.bacc as bacc
    import concourse.tile as tile
    from concourse import mybir
    from concourse.alu_op_type import AluOpType

    f32 = mybir.dt.float32
    bf16 = mybir.dt.bfloat16
    f8 = mybir.dt.float8e4
    DR = mybir.MatmulPerfMode.DoubleRow
    AF = mybir.ActivationFunctionType
    AX = mybir.AxisListType

    nc = bacc.Bacc("TRN2", target_bir_lowering=False, debug=False)

    # ---- DRAM parameters -------------------------------------------------
    d_obs = nc.dram_tensor("obs_pad", [n_scenes, 128, 16], bf16, kind="ExternalInput")
    d_twr = nc.dram_tensor("twr", [n_scenes, 2, 128, P * P], bf16, kind="ExternalInput")
    d_h = nc.dram_tensor("h_fm", [n_scenes, 64, P], bf16, kind="ExternalInput")
    d_Dm = nc.dram_tensor("Dm", [128, P * P], bf16, kind="ExternalInput")
    d_Wf = nc.dram_tensor("Wf_sb", [128, 2, D1], bf16, kind="ExternalInput")
    d_W1h = nc.dram_tensor("W1h_sb", [128, D1], bf16, kind="ExternalInput")
    d_W2h = nc.dram_tensor("W2h_sb", [128, 4, D2], f8, kind="ExternalInput")
    d_W2l = nc.dram_tensor("W2l_sb", [128, 4, D2], f8, kind="ExternalInput")
    d_b1 = nc.dram_tensor("b1_sb", [128, 4], f32, kind="ExternalInput")
    d_b2 = nc.dram_tensor("b2_sb", [128, 8], f32, kind="ExternalInput")
    d_id = nc.dram_tensor("ident", [128, 128], f32, kind="ExternalInput")
    d_out = nc.dram_tensor("out", [n_scenes * P, D2], f32, kind="ExternalOutput")

    with ExitStack() as ctx:
        tc = ctx.enter_context(tile.TileContext(nc))
        consts = ctx.enter_context(tc.tile_pool(name="consts", bufs=1))
        tw_pool = ctx.enter_context(tc.tile_pool(name="tw", bufs=2))
        scene_pool = ctx.enter_context(tc.tile_pool(name="scene", bufs=2))
        blk_pool = ctx.enter_context(tc.tile_pool(name="blk", bufs=3))
        pp = ctx.enter_context(tc.tile_pool(name="pp", bufs=1, space="PSUM"))
        p1 = ctx.enter_context(tc.tile_pool(name="p1", bufs=3, space="PSUM"))
        p2 = ctx.enter_context(tc.tile_pool(name="p2", bufs=4, space="PSUM"))

        # ---- small resident constants (big weights stream in after the
        # first scene's data so the first matmuls start sooner) ----------
        Dm_sb = consts.tile([128, P * P], bf16)
        b1_sb = consts.tile([128, 4], f32)
        b2_sb = consts.tile([128, 8], f32)
        id_sb = consts.tile([128, 128], f32)
        Wf_sb = consts.tile([128, 2, D1], bf16)
        W1h_sb = consts.tile([128, D1], bf16)
        W2h_sb = consts.tile([128, 4, D2], f8)
        W2l_sb = consts.tile([128, 4, D2], f8)

        def load_weights(twr0):
            # second wave, ordered by first use time (Sync queue), with all
            # small consts on the Scalar hwdge queue
            nc.scalar.dma_start(Wf_sb[:, 0], d_Wf[:, 0])
            nc.scalar.dma_start(Wf_sb[:, 1], d_Wf[:, 1])
            nc.scalar.dma_start(W1h_sb[:], d_W1h[:])
            nc.scalar.dma_start(b1_sb[:], d_b1[:])
            nc.scalar.dma_start(b2_sb[:], d_b2[:])
            nc.scalar.dma_start(id_sb[:], d_id[:])
            nc.sync.dma_start(Dm_sb[:, 2 * NBLK:], d_Dm[:, 2 * NBLK:])
            for half in range(2):
                nc.sync.dma_start(twr0[:, half, NBLK:3 * NBLK],
                                  d_twr[0, half, :, NBLK:3 * NBLK])
            for k in range(4):
                nc.sync.dma_start(W2h_sb[:, k], d_W2h[:, k])
            for half in range(2):
                nc.sync.dma_start(twr0[:, half, 3 * NBLK:],
                                  d_twr[0, half, :, 3 * NBLK:])
            for k in range(4):
                nc.sync.dma_start(W2l_sb[:, k], d_W2l[:, k])

        blocks = [(s, b) for s in range(n_scenes) for b in range(NB)]
        state = {}   # per-scene tiles
        mlp_q = []   # software pipeline: deferred MLP stage

        def scene_setup(s, first=False):
            twr = tw_pool.tile([128, 2, P * P], bf16, tag="twr")
            obs_c = scene_pool.tile([128, 16], bf16, tag="obs_c")
            h_c = scene_pool.tile([64, P], bf16, tag="h_c")
            if first:
                # Head is bandwidth+descriptor-gen bound: move only what the
                # first two blocks need first (obs, Dm rows 0:48, twr cols
                # 0:768), split across both hwdge queues (Sync + Activation);
                # the twr remainder and weights follow in load_weights().
                nc.sync.dma_start(obs_c[:], d_obs[s])
                nc.sync.dma_start(Dm_sb[:, :2 * NBLK], d_Dm[:, :2 * NBLK])
                nc.scalar.dma_start(h_c[:], d_h[s])
                for half in range(2):
                    nc.sync.dma_start(twr[:, half, :NBLK],
                                      d_twr[s, half, :, :NBLK])
            else:
                nc.sync.dma_start(twr[:, 0], d_twr[s, 0])
                nc.sync.dma_start(twr[:, 1], d_twr[s, 1])
                nc.sync.dma_start(obs_c[:], d_obs[s])
                nc.sync.dma_start(h_c[:], d_h[s])
            obs_rep = scene_pool.tile([128, 128], bf16, tag="obs_rep")
            nc.vector.tensor_copy(
                obs_rep[:].rearrange("p (r g) -> p r g", r=8),
                obs_c[:].unsqueeze(1).broadcast_to([128, 8, 16]))
            # h broadcast over i: hj_fm[:, ii*P + j] = h_fm[s, :, j];
            # K rows 64-127 are zero (W1h_sb rows are zero too, but NaN
            # garbage would still poison 0*NaN)
            hj_fm = scene_pool.tile([128, NBLK], bf16, tag="hj_fm")
            nc.gpsimd.memset(hj_fm[64:128, :], 0.0)
            nc.vector.tensor_copy(
                hj_fm[:64].rearrange("p (r j) -> p r j", r=IB),
                h_c[:].unsqueeze(1).broadcast_to([64, IB, P]))
            pooled = scene_pool.tile([128, 4, 2 * P], f32, tag="pooled")
            state[s] = dict(twr=twr, obs_rep=obs_rep, hj_fm=hj_fm, pooled=pooled)

        def prep(s, b):
            st = state[s]
            c0 = b * NBLK
            rel_ps = pp.tile([128, NBLK], f32, tag="pp")
            nc.tensor.matmul(rel_ps[:], st["obs_rep"][:],
                             Dm_sb[:, c0:c0 + NBLK], start=True, stop=True)
            rel2_0 = blk_pool.tile([128, NBLK], bf16, tag="rel2_0")
            nc.vector.tensor_tensor(rel2_0[:], st["twr"][:, 0, c0:c0 + NBLK],
                                    rel_ps[:], AluOpType.mult)
            rel2_1 = blk_pool.tile([128, NBLK], bf16, tag="rel2_1")
            nc.vector.tensor_tensor(rel2_1[:], st["twr"][:, 1, c0:c0 + NBLK],
                                    rel_ps[:], AluOpType.mult)
            return dict(rel2_0=rel2_0, rel2_1=rel2_1, s=s, b=b)

        def mlp1(job):
            s, b = job["s"], job["b"]
            st = state[s]
            r20 = job["rel2_0"][:]
            r21 = job["rel2_1"][:]
            # x1 in bf16 (for the residual) + fp8 hi + fp8 lo; MLP2 contracts
            # hi against W2 hi+lo and lo against W2 hi (DoubleRow fp8)
            x1b = blk_pool.tile([128, 4, NBLK], bf16, tag="x1b")
            x1h = blk_pool.tile([128, 4, NBLK], f8, tag="x1h")
            x1l = blk_pool.tile([128, 4, NBLK], f8, tag="x1l")
            for m in range(4):
                p1t = p1.tile([128, NBLK], f32, tag="p1")
                nc.tensor.matmul(p1t[:], Wf_sb[:, 0, m * 128:(m + 1) * 128],
                                 r20, start=True, stop=False)
                nc.tensor.matmul(p1t[:], Wf_sb[:, 1, m * 128:(m + 1) * 128],
                                 r21, start=False, stop=False)
                nc.tensor.matmul(p1t[:], W1h_sb[:, m * 128:(m + 1) * 128],
                                 st["hj_fm"][:], start=False, stop=True)
                nc.scalar.activation(x1b[:, m, :], p1t[:], AF.Relu,
                                     bias=b1_sb[:, m:m + 1])
                nc.scalar.activation(x1h[:, m, :], p1t[:], AF.Relu,
                                     bias=b1_sb[:, m:m + 1])
                nc.gpsimd.tensor_tensor(x1l[:, m, :], x1b[:, m, :],
                                        x1h[:, m, :], AluOpType.subtract)
            job["x1h"] = x1h
            job["x1l"] = x1l

        def mlp2(job):
            s, b = job["s"], job["b"]
            st = state[s]
            x1h = job["x1h"]
            x1l = job["x1l"]
            last = b == NB - 1
            for mm in range(8):
                ms = slice(mm * 128, (mm + 1) * 128)
                p2t = p2.tile([128, NBLK], f32, tag="p2")
                # 6 DoubleRow matmuls; consecutive pairs share lhsT so the
                # (unhidden part of) LDWEIGHTS amortizes
                nc.tensor.matmul(p2t[:], W2h_sb[:, 0:2, ms], x1h[:, 0:2, :],
                                 start=True, stop=False, perf_mode=DR)
                nc.tensor.matmul(p2t[:], W2h_sb[:, 0:2, ms], x1l[:, 0:2, :],
                                 start=False, stop=False, perf_mode=DR)
                nc.tensor.matmul(p2t[:], W2h_sb[:, 2:4, ms], x1h[:, 2:4, :],
                                 start=False, stop=False, perf_mode=DR)
                nc.tensor.matmul(p2t[:], W2h_sb[:, 2:4, ms], x1l[:, 2:4, :],
                                 start=False, stop=False, perf_mode=DR)
                nc.tensor.matmul(p2t[:], W2l_sb[:, 0:2, ms], x1h[:, 0:2, :],
                                 start=False, stop=False, perf_mode=DR)
                nc.tensor.matmul(p2t[:], W2l_sb[:, 2:4, ms], x1h[:, 2:4, :],
                                 start=False, stop=True, perf_mode=DR)
                dst = st["pooled"][:, mm // 2,
                                   (mm % 2) * P + b * IB:(mm % 2) * P + (b + 1) * IB]
                nc.vector.tensor_reduce(
                    dst, p2t[:].rearrange("p (i j) -> p i j", i=IB),
                    axis=AX.X, op=AluOpType.max)
                # one-group delay so PE doesn't stall on the pair's
                # reduce -> transpose chain (no delay on the final scene,
                # where it would only stretch the kernel tail)
                if last and mm % 2 == 1:
                    if s == n_scenes - 1:
                        finish_pair(s, st, mm // 2)
                    elif mm >= 3:
                        finish_pair(s, st, (mm - 3) // 2)
            if last:
                if s != n_scenes - 1:
                    finish_pair(s, st, 2)
                    finish_pair(s, st, 3)
                state.pop(s)

        def finish_pair(s, st, pi):
            """Scene output for m-tile pair pi: bias+relu post-pool on the
            Scalar engine, transpose to row-major, stage to SBUF, DMA out."""
            pooled = st["pooled"]
            fin = scene_pool.tile([128, 2 * P], f32, tag="fin")
            for half in range(2):
                mm = 2 * pi + half
                nc.scalar.activation(
                    fin[:, half * P:(half + 1) * P],
                    pooled[:, pi, half * P:(half + 1) * P],
                    AF.Relu, bias=b2_sb[:, mm:mm + 1], scale=1.0 / S_W)
            tps = p1.tile([128, NBLK], f32, tag="p1")
            nc.tensor.transpose(tps[:2 * P, :128], fin[:], id_sb[:])
            ot = scene_pool.tile([2 * P, 128], f32, tag="ot")
            nc.vector.tensor_copy(ot[:], tps[:2 * P, :128])
            nc.sync.dma_start(
                d_out[s * P:(s + 1) * P, (2 * pi) * 128:(2 * pi + 1) * 128],
                ot[:P, :])
            nc.sync.dma_start(
                d_out[s * P:(s + 1) * P, (2 * pi + 1) * 128:(2 * pi + 2) * 128],
                ot[P:2 * P, :])

        # two-deep software pipeline on PE:
        #   ... prep(i)  mlp1(i-1)  mlp2(i-2) ...
        # so x1 is ready a full block before MLP2 consumes it and PSUM
        # slot recycling has a block of slack; scene data is prefetched
        # one block before the scene starts
        scene_setup(0, first=True)
        for idx, (s, b) in enumerate(blocks):
            if b == NB - 2 and s + 1 < n_scenes:
                scene_setup(s + 1)
            if idx == 0:
                load_weights(state[0]["twr"])
            mlp_q.append(prep(s, b))
            if len(mlp_q) > 1:
                mlp1(mlp_q[-2])
            if len(mlp_q) > 2:
                mlp2(mlp_q.pop(0))
        mlp1(mlp_q[-1])
        mlp2(mlp_q.pop(0))
        mlp2(mlp_q.pop(0))

    nc.compile()
    return nc


def _host_inputs(h_states, traj, traj_weight, consts, n_scenes=SC):
    """Slice + lay out per-core input maps (matmul operands cast to bf16)."""
    import ml_dtypes
    bf = ml_dtypes.bfloat16
    h_states = np.asarray(h_states, np.float32)
    traj = np.asarray(traj, np.float32)
    traj_weight = np.asarray(traj_weight, np.float32)

    obs_full = np.ascontiguousarray(
        traj[:T].transpose(1, 0, 2).reshape(B, 2 * T))          # (B,16) g=t*2+c
    h_full = h_states.reshape(S, P, H)

    consts = dict(consts)
    for k in ("Wf_sb", "W1h_sb", "Dm"):
        consts[k] = consts[k].astype(bf)

    # traj_weight -> twT[s, ct, row] with ct = c*8+t, then pre-replicate
    # each ct row 16x onto partitions (two halves: ct 0-7 / ct 8-15)
    twT = np.ascontiguousarray(
        traj_weight.transpose(0, 2, 3, 1).reshape(S, 16, P * P)).astype(bf)
    twr = np.stack([np.repeat(twT[:, :8, :], 16, axis=1),
                    np.repeat(twT[:, 8:, :], 16, axis=1)], axis=1)  # (S,2,128,PP)

    in_maps = []
    for core in range(NCORES):
        s0 = core * n_scenes
        sl = slice(s0, s0 + n_scenes)
        h_fm = np.ascontiguousarray(h_full[sl].transpose(0, 2, 1)).astype(bf)
        obs_pad = np.zeros((n_scenes, 128, 16), np.float32)
        obs_pad[:, :P, :] = obs_full[s0 * P:(s0 + n_scenes) * P].reshape(
            n_scenes, P, 2 * T)
        m = dict(obs_pad=obs_pad.astype(bf), twr=np.ascontiguousarray(twr[sl]),
                 h_fm=h_fm)
        m.update(consts)
        in_maps.append(m)
    return in_maps


def kernel(h_states, seq_start_end, end_pos, traj, traj_weight,
           mlp_pre_pool_dim_0, W_se, b_se, W1, b1, W2, b2):
    import sys
    if '/opt/trn_rl_repo' not in sys.path:
        sys.path.insert(0, '/opt/trn_rl_repo')
    from concourse.bass_utils import run_bass_kernel_spmd

    consts = _host_constants(W_se, W1, W2, b1, b2)
    in_maps = _host_inputs(h_states, traj, traj_weight, consts)
    nc = build_program(SC)
    res = run_bass_kernel_spmd(nc, in_maps, list(range(NCORES)))
    out = np.concatenate([res.results[i]["out"] for i in range(NCORES)], axis=0)
    return out.astype(np.float32)


# revision 37
# speedup vs baseline: 1.3140x; 1.3140x over previous
# Trainium2 Bass kernel for nn_Decoder_14568529068506 (gnn_message_passing).
#
# Reference computation (per scene s of 32, P=48 peds):
#   rel[i,j]  = obs[j] - obs[i]                  (P,P,2T)   2T=16
#   emb       = rel @ W_se.T                     (P,P,512)
#   emb      *= tile(traj_weight[s])             (P,P,512)
#   x         = concat([emb, h[j]], -1)          (P,P,576)
#   x1        = relu(x @ W1.T + b1)              (P,P,512)
#   x2        = relu(x1 @ W2.T + b2)             (P,P,1024)
#   out[s,i]  = max_j x2[i,j]                    (P,1024)
#
# Kernel restructuring (validated exactly in fp32 numpy):
#  * The tiled traj_weight multiply + spatial embedding + W1 are fused:
#      out1[d,row] = sum_{(ct,g)} Wf[d,(ct,g)] * tw[row,ct] * rel[row,g]
#    with Wf[d, ct*16+g] = sum_{k%2==c} W1[d, t*64+k] * W_se[t*64+k, g].
#    So MLP1 contracts over 256 "rel2" features (+64 h features) instead
#    of 576, and the (P,P,512) embedding is never materialized.
#  * rel2 = tw_rep * rel_rep is built feature-major on 128 partitions:
#      rel_rep = obs_rep.T @ D   (D = +-1 pairwise difference matrix)
#    and tw_rep comes pre-replicated from the host via DMA (bandwidth is
#    cheap here; the replication matmuls caused PE pipeline stalls).
#  * ALL matmuls are full K=128 (zero-padded where needed): quadrant
#    (row_grp/tile_position) matmuls serialize LDWEIGHTS behind the
#    draining matmul (~+100ns each); full-tile matmuls chain at
#    163ns/384cols = 2.4GHz.
#  * relu/bias commute with max-pool, so MLP2 outputs are max-pooled
#    straight out of PSUM (reduces split across DVE and Pool engines);
#    bias+relu are applied post-pool on the Scalar engine.
#  * Matmuls run in bf16. PSUM accumulation stays fp32.
#
# Sharding: scenes are data-parallel across the 8 cores (4 scenes each);
# weights replicated; the (192,1024) per-core outputs are concatenated on
# the host (no collectives needed).

import numpy as np

S, P, T, E, H = 32, 48, 8, 64, 64
D1, D2 = 512, 1024
B = S * P
NCORES = 8
SC = S // NCORES          # scenes per core
NB = 6                    # row blocks per scene
NBLK = P * P // NB        # 384 columns (pairs) per block = 8 i-groups x 48 j
IB = NBLK // P            # i-groups per block (8)
S_W = 16.0                # fp8 scale on W2 (unapplied post-pool)


def _host_constants(W_se, W1, W2, b1, b2):
    """Precompute fused weights + structural constant matrices (fp32)."""
    W_se = np.asarray(W_se, np.float32)
    W1 = np.asarray(W1, np.float32)
    W2 = np.asarray(W2, np.float32)
    b1 = np.asarray(b1, np.float32)
    b2 = np.asarray(b2, np.float32)

    W1e, W1h = W1[:, :512], W1[:, 512:]
    Wf = np.zeros((D1, 256), np.float32)
    for c in range(2):
        for t in range(T):
            ct = c * 8 + t
            f = t * 64 + np.arange(c, 64, 2)
            Wf[:, ct * 16:(ct + 1) * 16] = W1e[:, f] @ W_se[f, :]

    # pairwise difference matrix, K zero-padded to 128
    Dm = np.zeros((128, P * P), np.float32)
    ii, jj = np.meshgrid(np.arange(P), np.arange(P), indexing="ij")
    rows = (ii * P + jj).ravel()
    np.add.at(Dm, (jj.ravel(), rows), 1.0)
    np.add.at(Dm, (ii.ravel(), rows), -1.0)

    # lhsT tile layouts: [128, kTiles, M] so DMAs are contiguous
    Wf_sb = np.ascontiguousarray(Wf.T.reshape(2, 128, D1).transpose(1, 0, 2))
    W1h_sb = np.zeros((128, D1), np.float32)
    W1h_sb[:H, :] = W1h.T                                    # K-padded
    W2_sb = np.ascontiguousarray(W2.T.reshape(4, 128, D2).transpose(1, 0, 2))
    b1_sb = np.ascontiguousarray(b1.reshape(4, 128).T)       # (128, 4)
    b2_sb = np.ascontiguousarray(b2.reshape(8, 128).T)       # (128, 8)
    ident = np.eye(128, dtype=np.float32)
    return dict(Wf_sb=Wf_sb, W1h_sb=W1h_sb, W2_sb=W2_sb, b1_sb=b1_sb,
                b2_sb=b2_sb, Dm=Dm, ident=ident)


def build_program(n_scenes=SC):
    """Emit the per-core Bass/Tile program. Returns the compiled Bacc."""
    from contextlib import ExitStack
    import concourse.bacc as bacc
    import concourse.tile as tile
    from concourse import mybir
    from concourse.alu_op_type import AluOpType

    f32 = mybir.dt.float32
    bf16 = mybir.dt.bfloat16
    f8 = mybir.dt.float8e4
    DR = mybir.MatmulPerfMode.DoubleRow
    AF = mybir.ActivationFunctionType
    AX = mybir.AxisListType

    nc = bacc.Bacc("TRN2", target_bir_lowering=False, debug=False)

    # ---- DRAM parameters -------------------------------------------------
    d_obs = nc.dram_tensor("obs_pad", [n_scenes, 128, 16], bf16, kind="ExternalInput")
    d_twr = nc.dram_tensor("twr", [n_scenes, 2, 128, P * P], bf16, kind="ExternalInput")
    d_h = nc.dram_tensor("h_fm", [n_scenes, 64, P], bf16, kind="ExternalInput")
    d_Dm = nc.dram_tensor("Dm", [128, P * P], bf16, kind="ExternalInput")
    d_Wf = nc.dram_tensor("Wf_sb", [128, 2, D1], bf16, kind="ExternalInput")
    d_W1h = nc.dram_tensor("W1h_sb", [128, D1], bf16, kind="ExternalInput")
    d_W2 = nc.dram_tensor("W2_sb", [128, 4, D2], bf16, kind="ExternalInput")
    d_b1 = nc.dram_tensor("b1_sb", [128, 4], f32, kind="ExternalInput")
    d_b2 = nc.dram_tensor("b2_sb", [128, 8], f32, kind="ExternalInput")
    d_id = nc.dram_tensor("ident", [128, 128], f32, kind="ExternalInput")
    d_out = nc.dram_tensor("out", [n_scenes * P, D2], f32, kind="ExternalOutput")

    with ExitStack() as ctx:
        tc = ctx.enter_context(tile.TileContext(nc))
        consts = ctx.enter_context(tc.tile_pool(name="consts", bufs=1))
        tw_pool = ctx.enter_context(tc.tile_pool(name="tw", bufs=2))
        scene_pool = ctx.enter_context(tc.tile_pool(name="scene", bufs=2))
        blk_pool = ctx.enter_context(tc.tile_pool(name="blk", bufs=3))
        pp = ctx.enter_context(tc.tile_pool(name="pp", bufs=1, space="PSUM"))
        p1 = ctx.enter_context(tc.tile_pool(name="p1", bufs=3, space="PSUM"))
        p2 = ctx.enter_context(tc.tile_pool(name="p2", bufs=4, space="PSUM"))

        # ---- small resident constants (big weights stream in after the
        # first scene's data so the first matmuls start sooner) ----------
        Dm_sb = consts.tile([128, P * P], bf16)
        b1_sb = consts.tile([128, 4], f32)
        b2_sb = consts.tile([128, 8], f32)
        id_sb = consts.tile([128, 128], f32)
        Wf_sb = consts.tile([128, 2, D1], bf16)
        W1h_sb = consts.tile([128, D1], bf16)
        W2_sb = consts.tile([128, 4, D2], bf16)

        def load_weights(twr0):
            # second wave, ordered by first use time (Sync queue), with all
            # small consts on the Scalar hwdge queue
            nc.scalar.dma_start(Wf_sb[:, 0], d_Wf[:, 0])
            nc.scalar.dma_start(Wf_sb[:, 1], d_Wf[:, 1])
            nc.scalar.dma_start(W1h_sb[:], d_W1h[:])
            nc.scalar.dma_start(b1_sb[:], d_b1[:])
            nc.scalar.dma_start(b2_sb[:], d_b2[:])
            nc.scalar.dma_start(id_sb[:], d_id[:])
            nc.sync.dma_start(Dm_sb[:, 2 * NBLK:], d_Dm[:, 2 * NBLK:])
            for half in range(2):
                nc.sync.dma_start(twr0[:, half, 2 * NBLK:4 * NBLK],
                                  d_twr[0, half, :, 2 * NBLK:4 * NBLK])
            for k in range(4):
                nc.sync.dma_start(W2_sb[:, k], d_W2[:, k])
            for half in range(2):
                nc.sync.dma_start(twr0[:, half, 4 * NBLK:],
                                  d_twr[0, half, :, 4 * NBLK:])

        blocks = [(s, b) for s in range(n_scenes) for b in range(NB)]
        state = {}   # per-scene tiles
        mlp_q = []   # software pipeline: deferred MLP stage

        def scene_setup(s, first=False):
            twr = tw_pool.tile([128, 2, P * P], bf16, tag="twr")
            obs_c = scene_pool.tile([128, 16], bf16, tag="obs_c")
            h_c = scene_pool.tile([64, P], bf16, tag="h_c")
            if first:
                # Head is bandwidth+descriptor-gen bound: move only what the
                # first two blocks need first (obs, Dm rows 0:48, twr cols
                # 0:768), split across both hwdge queues (Sync + Activation);
                # the twr remainder and weights follow in load_weights().
                nc.sync.dma_start(obs_c[:], d_obs[s])
                nc.sync.dma_start(Dm_sb[:, :2 * NBLK], d_Dm[:, :2 * NBLK])
                nc.scalar.dma_start(h_c[:], d_h[s])
                for half in range(2):
                    nc.sync.dma_start(twr[:, half, :2 * NBLK],
                                      d_twr[s, half, :, :2 * NBLK])
            else:
                nc.sync.dma_start(twr[:, 0], d_twr[s, 0])
                nc.sync.dma_start(twr[:, 1], d_twr[s, 1])
                nc.sync.dma_start(obs_c[:], d_obs[s])
                nc.sync.dma_start(h_c[:], d_h[s])
            obs_rep = scene_pool.tile([128, 128], bf16, tag="obs_rep")
            nc.vector.tensor_copy(
                obs_rep[:].rearrange("p (r g) -> p r g", r=8),
                obs_c[:].unsqueeze(1).broadcast_to([128, 8, 16]))
            # h broadcast over i: hj_fm[:, ii*P + j] = h_fm[s, :, j];
            # K rows 64-127 are zero (W1h_sb rows are zero too, but NaN
            # garbage would still poison 0*NaN)
            hj_fm = scene_pool.tile([128, NBLK], bf16, tag="hj_fm")
            nc.gpsimd.memset(hj_fm[64:128, :], 0.0)
            nc.vector.tensor_copy(
                hj_fm[:64].rearrange("p (r j) -> p r j", r=IB),
                h_c[:].unsqueeze(1).broadcast_to([64, IB, P]))
            pooled = scene_pool.tile([128, 4, 2 * P], f32, tag="pooled")
            state[s] = dict(twr=twr, obs_rep=obs_rep, hj_fm=hj_fm, pooled=pooled)

        def prep(s, b):
            st = state[s]
            c0 = b * NBLK
            rel_ps = pp.tile([128, NBLK], f32, tag="pp")
            nc.tensor.matmul(rel_ps[:], st["obs_rep"][:],
                             Dm_sb[:, c0:c0 + NBLK], start=True, stop=True)
            rel2_0 = blk_pool.tile([128, NBLK], bf16, tag="rel2_0")
            nc.vector.tensor_tensor(rel2_0[:], st["twr"][:, 0, c0:c0 + NBLK],
                                    rel_ps[:], AluOpType.mult)
            rel2_1 = blk_pool.tile([128, NBLK], bf16, tag="rel2_1")
            nc.vector.tensor_tensor(rel2_1[:], st["twr"][:, 1, c0:c0 + NBLK],
                                    rel_ps[:], AluOpType.mult)
            return dict(rel2_0=rel2_0, rel2_1=rel2_1, s=s, b=b)

        def mlp1(job):
            s, b = job["s"], job["b"]
            st = state[s]
            r20 = job["rel2_0"][:]
            r21 = job["rel2_1"][:]
            x1 = blk_pool.tile([128, 4, NBLK], bf16, tag="x1")
            for m in range(4):
                p1t = p1.tile([128, NBLK], f32, tag="p1")
                nc.tensor.matmul(p1t[:], Wf_sb[:, 0, m * 128:(m + 1) * 128],
                                 r20, start=True, stop=False)
                nc.tensor.matmul(p1t[:], Wf_sb[:, 1, m * 128:(m + 1) * 128],
                                 r21, start=False, stop=False)
                nc.tensor.matmul(p1t[:], W1h_sb[:, m * 128:(m + 1) * 128],
                                 st["hj_fm"][:], start=False, stop=True)
                nc.scalar.activation(x1[:, m, :], p1t[:], AF.Relu,
                                     bias=b1_sb[:, m:m + 1])
            job["x1"] = x1

        def mlp2(job):
            s, b = job["s"], job["b"]
            st = state[s]
            x1 = job["x1"]
            last = b == NB - 1
            for mm in range(8):
                p2t = p2.tile([128, NBLK], f32, tag="p2")
                for k in range(4):
                    nc.tensor.matmul(
                        p2t[:], W2_sb[:, k, mm * 128:(mm + 1) * 128],
                        x1[:, k, :], start=(k == 0), stop=(k == 3))
                dst = st["pooled"][:, mm // 2,
                                   (mm % 2) * P + b * IB:(mm % 2) * P + (b + 1) * IB]
                nc.vector.tensor_reduce(
                    dst, p2t[:].rearrange("p (i j) -> p i j", i=IB),
                    axis=AX.X, op=AluOpType.max)
                # one-group delay so PE doesn't stall on the pair's
                # reduce -> transpose chain (no delay on the final scene,
                # where it would only stretch the kernel tail)
                if last and mm % 2 == 1:
                    if s == n_scenes - 1:
                        finish_pair(s, st, mm // 2)
                    elif mm >= 3:
                        finish_pair(s, st, (mm - 3) // 2)
            if last:
                if s != n_scenes - 1:
                    finish_pair(s, st, 2)
                    finish_pair(s, st, 3)
                state.pop(s)

        def finish_pair(s, st, pi):
            """Scene output for m-tile pair pi: bias+relu post-pool on the
            Scalar engine, transpose to row-major, stage to SBUF, DMA out."""
            pooled = st["pooled"]
            fin = scene_pool.tile([128, 2 * P], f32, tag="fin")
            for half in range(2):
                mm = 2 * pi + half
                nc.scalar.activation(
                    fin[:, half * P:(half + 1) * P],
                    pooled[:, pi, half * P:(half + 1) * P],
                    AF.Relu, bias=b2_sb[:, mm:mm + 1])
            tps = p1.tile([128, NBLK], f32, tag="p1")
            nc.tensor.transpose(tps[:2 * P, :128], fin[:], id_sb[:])
            ot = scene_pool.tile([2 * P, 128], f32, tag="ot")
            nc.vector.tensor_copy(ot[:], tps[:2 * P, :128])
            nc.sync.dma_start(
                d_out[s * P:(s + 1) * P, (2 * pi) * 128:(2 * pi + 1) * 128],
                ot[:P, :])
            nc.sync.dma_start(
                d_out[s * P:(s + 1) * P, (2 * pi + 1) * 128:(2 * pi + 2) * 128],
                ot[P:2 * P, :])

        # two-deep software pipeline on PE:
        #   ... prep(i)  mlp1(i-1)  mlp2(i-2) ...
        # so x1 is ready a full block before MLP2 consumes it and PSUM
        # slot recycling has a block of slack; scene data is prefetched
        # one block before the scene starts
        scene_setup(0, first=True)
        # warm-up: ramp the PE p-state (2.4GHz needs ~3us of continuous
        # execution) on zeros while the first DMA wave is in flight
        warm = consts.tile([128, NBLK], bf16)
        nc.gpsimd.memset(warm[:], 0.0)
        for w in range(8):
            wps = p2.tile([128, NBLK], f32, tag="p2")
            nc.tensor.matmul(wps[:], warm[:, :128], warm[:], start=True,
                             stop=True)
        for idx, (s, b) in enumerate(blocks):
            if b == NB - 2 and s + 1 < n_scenes:
                scene_setup(s + 1)
            if idx == 0:
                load_weights(state[0]["twr"])
            mlp_q.append(prep(s, b))
            if len(mlp_q) > 1:
                mlp1(mlp_q[-2])
            if len(mlp_q) > 2:
                mlp2(mlp_q.pop(0))
        mlp1(mlp_q[-1])
        mlp2(mlp_q.pop(0))
        mlp2(mlp_q.pop(0))

    nc.compile()
    return nc


def _host_inputs(h_states, traj, traj_weight, consts, n_scenes=SC):
    """Slice + lay out per-core input maps (matmul operands cast to bf16)."""
    import ml_dtypes
    bf = ml_dtypes.bfloat16
    h_states = np.asarray(h_states, np.float32)
    traj = np.asarray(traj, np.float32)
    traj_weight = np.asarray(traj_weight, np.float32)

    obs_full = np.ascontiguousarray(
        traj[:T].transpose(1, 0, 2).reshape(B, 2 * T))          # (B,16) g=t*2+c
    h_full = h_states.reshape(S, P, H)

    consts = dict(consts)
    for k in ("Wf_sb", "W1h_sb", "W2_sb", "Dm"):
        consts[k] = consts[k].astype(bf)

    # traj_weight -> twT[s, ct, row] with ct = c*8+t, then pre-replicate
    # each ct row 16x onto partitions (two halves: ct 0-7 / ct 8-15)
    twT = np.ascontiguousarray(
        traj_weight.transpose(0, 2, 3, 1).reshape(S, 16, P * P)).astype(bf)
    twr = np.stack([np.repeat(twT[:, :8, :], 16, axis=1),
                    np.repeat(twT[:, 8:, :], 16, axis=1)], axis=1)  # (S,2,128,PP)

    in_maps = []
    for core in range(NCORES):
        s0 = core * n_scenes
        sl = slice(s0, s0 + n_scenes)
        h_fm = np.ascontiguousarray(h_full[sl].transpose(0, 2, 1)).astype(bf)
        obs_pad = np.zeros((n_scenes, 128, 16), np.float32)
        obs_pad[:, :P, :] = obs_full[s0 * P:(s0 + n_scenes) * P].reshape(
            n_scenes, P, 2 * T)
        m = dict(obs_pad=obs_pad.astype(bf), twr=np.ascontiguousarray(twr[sl]),
                 h_fm=h_fm)
        m.update(consts)
        in_maps.append(m)
    return in_maps


def kernel(h_states, seq_start_end, end_pos, traj, traj_weight,
           mlp_pre_pool_dim_0, W_se, b_se, W1, b1, W2, b2):
    import sys
    if '/opt/trn_rl_repo' not in sys.path:
        sys.path.insert(0, '/opt/trn_rl_repo')
    from concourse.bass_utils import run_bass_kernel_spmd

    consts = _host_constants(W_se, W1, W2, b1, b2)
    in_maps = _host_inputs(h_states, traj, traj_weight, consts)
    nc = build_program(SC)
    res = run_bass_kernel_spmd(nc, in_maps, list(range(NCORES)))
    out = np.concatenate([res.results[i]["out"] for i in range(NCORES)], axis=0)
    return out.astype(np.float32)
